# revision 1
# baseline (speedup 1.0000x reference)
"""Multi-head attention (B=4, N=2048, C=1024, H=16) on 8 TRN2 NeuronCores.

Sharding: (batch, query-half) grid -> 8 cores, zero collectives.
Core c handles batch b = c//2, query chunk s = c%2 (1024 queries).
Each core computes K/V for all 2048 tokens of its batch (duplicated across
the 2 cores of a batch), attention for its 1024 queries x all 16 heads, and
the output projection for its query chunk. Outputs are disjoint slices of y.

Token-roll trick: the host passes x^T with token columns rolled so that the
core's own query half is always columns [0, 1024) -> identical SPMD graph on
all cores. Softmax/AV are permutation-invariant in key order, so the rolled
key order does not change results.

Layouts (SBUF, bf16 storage, f32 PSUM accumulation):
  xT  [c, tok]      Q^T/K^T [(h,d), tok]   V [tok, h*(HD+1)] ones-widened
  S^T [k, q] per (head, k-tile) -> exp on ACT -> P^T bf16 -> AV matmul
  O^T [(h,d), q] -> proj with host-transposed w_proj^T, bias via ones-row MM.
Softmax without max-subtraction (scores bounded for this distribution);
denominator comes free from the ones column of V (AV row 64 = sum_k P);
1/denom broadcast across partitions with a contract=1 ones matmul.

Engine discipline: PE/ACT/DVE + nc.sync DMAs only (gpsimd would push the
Tile tail-drain past walrus's per-instruction sync-wait limit).
"""

import sys

for _p in ("/opt/trn_rl_repo",):
    if _p not in sys.path:
        sys.path.insert(0, _p)

import numpy as np
import ml_dtypes

import bass_rust
import concourse.bass as bass
import concourse.mybir as mybir
import concourse.tile as tile
from concourse.bass_utils import run_bass_kernel_spmd
from concourse.vector_clock import ScopedClock


# --- tail-drain wait splitting -------------------------------------------
# Walrus codegen (CoreV3GenImpl setupSyncWait) rejects CTRL-class
# instructions carrying more than a few sync waits; Tile's kernel-tail drain
# waits on every engine/DMA-queue proc used (7 here), which fails codegen.
# Split the waits across the drain plus follow-up sync-engine NOPs emitted
# before the end-of-kernel barrier — semantically identical.
_WAIT_CHUNK = 1


def _split_drain_and_barrier(self, tick_clock, wait_clock):
    drain_inst = self.nc.sync.drain()
    wait_clock.add_sem_waits(
        drain_inst.ins, ScopedClock({None: tick_clock.global_clock})
    )
    si = drain_inst.ins.sync_info
    waits = list(si.on_wait) if si is not None and si.on_wait else []
    if len(waits) > _WAIT_CHUNK:
        si.on_wait = waits[:_WAIT_CHUNK]
        rest = waits[_WAIT_CHUNK:]
        while rest:
            take, rest = rest[:_WAIT_CHUNK], rest[_WAIT_CHUNK:]
            nop = self.nc.sync.nop(nofuse=True, hint="drain_split")
            nop.ins.sync_info = bass_rust.SyncInfo(on_wait=take, on_update=[])
    self.nc.all_engine_barrier()
    popped = self.nc._tile_sem_poison_stack.pop()
    assert popped is self._sem_poison
    self.nc.clear_and_free_semaphores(list(self.sems.allocated().values()))
    self.nc.all_engine_barrier()


tile.TileContext._drain_and_barrier = _split_drain_and_barrier


def _split_multi_waits(nc, limit=1):
    """Hoist all but `limit` sync waits of every instruction onto preceding
    same-engine NOPs (this walrus rejects >1 wait on any instruction)."""
    n = 0
    for f in nc.m.functions:
        for bb in f.blocks:
            new_insts = []
            for ins in bb.instructions:
                si = ins.sync_info
                waits = list(si.on_wait) if si is not None and si.on_wait else []
                if len(waits) > limit and ins.engine not in (
                    None,
                    mybir.EngineType.Unassigned,
                ):
                    for w in waits[:-limit]:
                        nop = mybir.InstNoOp(
                            name=f"{ins.name}.wsplit{n}", ins=[], outs=[]
                        )
                        n += 1
                        nop.engine = ins.engine
                        nop.sync_info = bass_rust.SyncInfo(
                            on_wait=[w], on_update=[]
                        )
                        nc.register_instruction(nop, overwrite=True)
                        new_insts.append(nop)
                    si.on_wait = waits[-limit:]
                new_insts.append(ins)
            bb.instructions = new_insts
    return n

BF16 = mybir.dt.bfloat16
F32 = mybir.dt.float32
NPBF16 = ml_dtypes.bfloat16

B, N, C = 4, 2048, 1024
H, HD = 16, 64
SCALE = HD**-0.5
NQ = N // 2          # local queries per core
P = 128              # partitions
CCH = C // P         # 8 contract chunks
NKT = N // P         # 16 key tiles
HP = H // 2          # 8 head pairs
QG = 512             # matmul free-dim group
VW = HD + 1          # V widened with ones column

_CACHE = {}


def _build():
    nc = bass.Bass()

    xT_d = nc.declare_dram_parameter("xT", [C, N], BF16, isOutput=False)
    wqkT_d = nc.declare_dram_parameter(
        "wqkT", [2 * CCH, P, CCH, P], BF16, isOutput=False
    )  # host pre-packed: [row-group, c-part, c-chunk, row]
    wvT_d = nc.declare_dram_parameter("wvT", [C, C], BF16, isOutput=False)
    wpT_d = nc.declare_dram_parameter("wpT", [C, C], BF16, isOutput=False)
    bias_d = nc.declare_dram_parameter("bias", [1, C], BF16, isOutput=False)
    y_d = nc.declare_dram_parameter("y", [NQ, C], F32, isOutput=True)

    xT_v = xT_d[:].rearrange("(cc p) n -> cc p n", p=P)        # [8,128,2048]
    wvT_v = wvT_d[:].rearrange("(cc p) r -> cc p r", p=P)      # [8,128,1024]
    wpT_v = wpT_d[:].rearrange("(cc p) r -> cc p r", p=P)      # [8,128,1024]

    with tile.TileContext(nc) as tc:
        with (
            tc.tile_pool(name="big", bufs=1) as big,
            tc.tile_pool(name="consts", bufs=1) as consts,
            tc.tile_pool(name="wstream", bufs=3) as wstream,
            tc.tile_pool(name="npool", bufs=2) as npool,
            tc.tile_pool(name="ypool", bufs=2) as ypool,
            tc.tile_pool(name="ppool", bufs=3) as ppool,
            tc.tile_pool(name="stashp", bufs=1) as stashp,
            tc.tile_pool(name="psmain", bufs=2, space="PSUM") as psmain,
            tc.tile_pool(name="psav", bufs=2, space="PSUM") as psav,
        ):
            # ---- resident SBUF tensors ----
            xT_sb = big.tile([P, CCH, N], BF16, tag="xT")
            qT_sb = big.tile([P, CCH, NQ], BF16, tag="qT")
            kT_sb = big.tile([P, CCH, N], BF16, tag="kT")
            v_sb = big.tile([P, NKT, H * VW], BF16, tag="v")
            oT_sb = big.tile([P, CCH, NQ], BF16, tag="oT")
            wpT_sb = big.tile([P, CCH, C], BF16, tag="wpT")
            wv_sb = big.tile([P, CCH, C], BF16, tag="wv")
            bias_sb = consts.tile([1, C], BF16, tag="bias")
            ones_sb = consts.tile([1, P], BF16, tag="ones")

            nc.vector.memset(ones_sb[:], 1.0)
            v_ones = v_sb[:].rearrange("p t (h e) -> p t h e", e=VW)[
                :, :, :, HD : HD + 1
            ]
            nc.vector.memset(v_ones, 1.0)

            nc.sync.dma_start(bias_sb[:], bias_d[:])
            # split the startup-critical x^T load across both HWDGE engine
            # front-ends (SP + ACT = 8 hardware queues); ACT is idle here
            for cc in range(CCH):
                eng = nc.sync if cc % 2 == 0 else nc.scalar
                eng.dma_start(xT_sb[:, cc, :], xT_v[cc])

            # ---- qkv production quanta ----
            def load_qk_slab(rg):
                """DMA 128 rows of w_qk^T (columns rg*128..) as [c-part, cc, row]."""
                wslab = wstream.tile([P, CCH, P], BF16, tag="wqk")
                nc.sync.dma_start(wslab[:], wqkT_d[rg])
                return wslab

            def qk_quantum(rg, wslab, tg):
                """One accumulation group: 128 rows x 512 tokens of Q^T or K^T."""
                dst = qT_sb if rg < CCH else kT_sb
                ch = rg % CCH
                ps = psmain.tile([P, 2 * QG], F32, tag="ps")
                for cc in range(CCH):
                    nc.tensor.matmul(
                        ps[:, 0:QG],
                        lhsT=wslab[:, cc, :],
                        rhs=xT_sb[:, cc, tg * QG : (tg + 1) * QG],
                        start=(cc == 0),
                        stop=(cc == CCH - 1),
                    )
                nc.vector.tensor_copy(
                    dst[:, ch, tg * QG : (tg + 1) * QG], ps[:, 0:QG]
                )

            def v_quantum(tc_i, vg):
                """V rows for tokens [tc_i*128, ..), head group vg, widened layout."""
                ps = psmain.tile([P, 2 * QG], F32, tag="ps")
                for cc in range(CCH):
                    nc.tensor.matmul(
                        ps[:, 0:QG],
                        lhsT=xT_sb[:, cc, tc_i * P : (tc_i + 1) * P],
                        rhs=wv_sb[:, cc, vg * QG : (vg + 1) * QG],
                        start=(cc == 0),
                        stop=(cc == CCH - 1),
                    )
                dst = v_sb[:, tc_i, vg * 8 * VW : (vg + 1) * 8 * VW].rearrange(
                    "p (h e) -> p h e", e=VW
                )[:, :, 0:HD]
                nc.vector.tensor_copy(
                    dst, ps[:, 0:QG].rearrange("p (h e) -> p h e", e=HD)
                )

            # pair 0's rows up front
            slab_q = load_qk_slab(0)
            slab_k0 = load_qk_slab(CCH)
            for cc in range(CCH):
                nc.scalar.dma_start(wv_sb[:, cc, :], wvT_v[cc])
            for tg in range(NQ // QG):
                qk_quantum(0, slab_q, tg)
            for tg in range(N // QG):
                qk_quantum(CCH, slab_k0, tg)

            # ---- attention over head pairs, with qkv work interleaved ----
            def normalize_from_stash(h, stash):
                """oT[h rows] = stash[0:64] * (1/stash[64]) broadcast."""
                recip_bf = npool.tile([1, NQ], BF16, tag="recipbf", name=f"recip_{h}")
                with nc.allow_low_precision(reason="softmax denom recip"):
                    nc.vector.reciprocal(recip_bf[:], stash[HD : HD + 1, :])
                bc = psmain.tile([P, 2 * QG], F32, tag="ps", name=f"bc_{h}")
                for qg in range(NQ // QG):
                    nc.tensor.matmul(
                        bc[0:HD, qg * QG : (qg + 1) * QG],
                        lhsT=ones_sb[0:1, 0:HD],
                        rhs=recip_bf[0:1, qg * QG : (qg + 1) * QG],
                        start=True,
                        stop=True,
                    )
                bc_sb = npool.tile([HD, NQ], F32, tag="bcsb", name=f"bcsb_{h}")
                nc.vector.tensor_copy(bc_sb[:], bc[0:HD, :])
                base = (h % 2) * HD
                nc.vector.tensor_mul(
                    oT_sb[base : base + HD, h // 2, :], stash[0:HD, :], bc_sb[:]
                )

            deferred = []
            for hp in range(HP):
                ha, hb = 2 * hp, 2 * hp + 1

                # pending work to interleave into this pair's kt loop:
                pending = []
                if hp == 0:
                    # V for token chunk kt is produced just before AV needs it,
                    # handled inline below; nothing else pending.
                    pass
                if hp + 1 < HP:
                    nslab_q = load_qk_slab(hp + 1)
                    nslab_k = load_qk_slab(CCH + hp + 1)
                    for tg in range(NQ // QG):
                        pending.append((hp + 1, nslab_q, tg))
                    for tg in range(N // QG):
                        pending.append((CCH + hp + 1, nslab_k, tg))

                if hp == 1:
                    # wpT is first needed by proj; load it in this quiet window
                    for cc in range(CCH):
                        nc.sync.dma_start(wpT_sb[:, cc, :], wpT_v[cc])

                av = {
                    h: psav.tile([VW, NQ], F32, tag="av", name=f"av_{h}")
                    for h in (ha, hb)
                }
                for kt in range(NKT):
                    # V head-group 0 feeds pair 0's AV just-in-time; head
                    # group 1 is first consumed at pair 4, so produce it
                    # during pair 1 to flatten the PE load profile.
                    if hp == 0:
                        v_quantum(kt, 0)
                    elif hp == 1 and kt % 2 == 0:
                        v_quantum(kt // 2, 1)
                    elif hp == 2 and kt % 2 == 0:
                        v_quantum(8 + kt // 2, 1)
                    for h in (ha, hb):
                        base = (h % 2) * HD
                        st = psmain.tile([P, 2 * QG], F32, tag="ps")
                        for qg in range(NQ // QG):
                            nc.tensor.matmul(
                                st[:, qg * QG : (qg + 1) * QG],
                                lhsT=kT_sb[
                                    base : base + HD, h // 2, kt * P : (kt + 1) * P
                                ],
                                rhs=qT_sb[
                                    base : base + HD, h // 2, qg * QG : (qg + 1) * QG
                                ],
                                start=True,
                                stop=True,
                            )
                        pt = ppool.tile([P, NQ], BF16, tag="p")
                        nc.scalar.activation(
                            pt[:],
                            st[:],
                            mybir.ActivationFunctionType.Exp,
                            scale=float(SCALE),
                        )
                        for qg in range(NQ // QG):
                            nc.tensor.matmul(
                                av[h][:, qg * QG : (qg + 1) * QG],
                                lhsT=v_sb[:, kt, h * VW : (h + 1) * VW],
                                rhs=pt[:, qg * QG : (qg + 1) * QG],
                                start=(kt == 0),
                                stop=(kt == NKT - 1),
                            )
                    # previous pair's deferred normalize runs in this pair's
                    # slack (its stash is SBUF; av slots were already freed)
                    if deferred and kt in (4, 5):
                        normalize_from_stash(*deferred.pop(0))
                    # interleave ~one pending qkv quantum every other kt,
                    # keeping one for the stash window after the loop
                    if len(pending) > 1 and kt >= 3 and kt % 2 == 1:
                        rg, slab, tg = pending.pop(0)
                        qk_quantum(rg, slab, tg)

                # drain remaining pending quanta
                for rg, slab, tg in pending:
                    qk_quantum(rg, slab, tg)

                # stash unnormalized AV output (+ denominator row) to SBUF so
                # the PSUM accumulator slots recycle immediately; normalize is
                # deferred into the next pair's kt loop (except the last pair,
                # which the projection depends on).
                if hp == HP - 1:
                    # no deferral benefit on the last pair: normalizing
                    # straight from PSUM shortens the projection-gating path
                    normalize_from_stash(ha, av[ha])
                    normalize_from_stash(hb, av[hb])
                else:
                    for h in (ha, hb):
                        stash = stashp.tile(
                            [VW, NQ], BF16, tag=f"stash{h % 2}", name=f"stash_{h}"
                        )
                        nc.vector.tensor_copy(stash[:], av[h][:])
                        deferred.append((h, stash))

            # ---- output projection ----
            for tc_i in range(NQ // P):
                for og in range(C // QG):
                    ps = psmain.tile([P, 2 * QG], F32, tag="ps")
                    for cc in range(CCH):
                        nc.tensor.matmul(
                            ps[:, 0:QG],
                            lhsT=oT_sb[:, cc, tc_i * P : (tc_i + 1) * P],
                            rhs=wpT_sb[:, cc, og * QG : (og + 1) * QG],
                            start=(cc == 0),
                            stop=False,
                        )
                    nc.tensor.matmul(
                        ps[:, 0:QG],
                        lhsT=ones_sb[0:1, 0:P],
                        rhs=bias_sb[0:1, og * QG : (og + 1) * QG],
                        start=False,
                        stop=True,
                    )
                    y_sb = ypool.tile([P, QG], F32, tag="ysb")
                    nc.vector.tensor_copy(y_sb[:], ps[:, 0:QG])
                    # ACT is idle by the projection tail; alternate both HWDGE
                    # front-ends so the 4MB output drains over 8 queues
                    eng = nc.sync if (tc_i + og) % 2 == 0 else nc.scalar
                    eng.dma_start(
                        y_d[tc_i * P : (tc_i + 1) * P, og * QG : (og + 1) * QG],
                        y_sb[:],
                    )
    _split_multi_waits(nc)
    return nc


def get_nc():
    if "nc" not in _CACHE:
        _CACHE["nc"] = _build()
    return _CACHE["nc"]


def make_in_maps(x, w_qkv, w_proj, b_proj):
    x = np.asarray(x, np.float32)
    w_qkv = np.asarray(w_qkv, np.float32)
    w_proj = np.asarray(w_proj, np.float32)
    b_proj = np.asarray(b_proj, np.float32)
    # pre-pack w_qk^T as [row-group, c-part, c-chunk, row] so slab DMAs are
    # fully contiguous per partition
    wqkT = np.ascontiguousarray(
        w_qkv[: 2 * C]
        .T.reshape(CCH, P, 2 * CCH, P)
        .transpose(2, 1, 0, 3)
    ).astype(NPBF16)
    wvT = np.ascontiguousarray(w_qkv[2 * C :].T).astype(NPBF16)
    wpT = np.ascontiguousarray(w_proj.T).astype(NPBF16)
    bias = b_proj.reshape(1, C).astype(NPBF16)
    in_maps = []
    for c in range(8):
        b, s = divmod(c, 2)
        xb = x[b].astype(NPBF16)  # [N, C]
        rolled = np.concatenate(
            [xb[s * NQ : (s + 1) * NQ], xb[(1 - s) * NQ : (2 - s) * NQ]], 0
        )
        xT = np.ascontiguousarray(rolled.T)  # [C, N], local queries first
        in_maps.append({"xT": xT, "wqkT": wqkT, "wvT": wvT, "wpT": wpT, "bias": bias})
    return in_maps


def kernel(x, w_qkv, w_proj, b_proj, _res_out=None):
    nc = get_nc()
    in_maps = make_in_maps(x, w_qkv, w_proj, b_proj)
    res = run_bass_kernel_spmd(nc, in_maps, core_ids=list(range(8)))
    if _res_out is not None:
        _res_out.append(res)

    y = np.empty((B, N, C), np.float32)
    for c in range(8):
        b, s = divmod(c, 2)
        y[b, s * NQ : (s + 1) * NQ] = res.results[c]["y"]
    return y


if __name__ == "__main__":
    rng = np.random.default_rng(0)
    inp = {
        "x": rng.standard_normal((B, N, C), dtype=np.float32),
        "w_qkv": rng.standard_normal((3 * C, C), dtype=np.float32) * C**-0.5,
        "w_proj": rng.standard_normal((C, C), dtype=np.float32) * C**-0.5,
        "b_proj": rng.standard_normal(C, dtype=np.float32) * 0.01,
    }
    y = kernel(**inp)
    print("ran", y.shape, y.dtype)



# revision 2
# speedup vs baseline: 1.1363x; 1.1363x over previous
"""Multi-head attention (B=4, N=2048, C=1024, H=16) on 8 TRN2 NeuronCores.

Sharding: (batch, query-half) grid -> 8 cores, zero collectives.
Core c handles batch b = c//2, query chunk s = c%2 (1024 queries).
Each core computes K/V for all 2048 tokens of its batch (duplicated across
the 2 cores of a batch), attention for its 1024 queries x all 16 heads, and
the output projection for its query chunk. Outputs are disjoint slices of y.

Token-roll trick: the host passes x^T with token columns rolled so that the
core's own query half is always columns [0, 1024) -> identical SPMD graph on
all cores. Softmax/AV are permutation-invariant in key order, so the rolled
key order does not change results.

v2 schedule: the ACT engine's exp is a hard ~294us floor (256 tiles x
(1024+352)cyc @ 1.2GHz), so the whole kernel is organized as a software
pipeline that never lets ACT wait:
  - queries processed in two sequential 512-halves per head pair, so the
    AV accumulators shrink to 1 PSUM bank each; PSUM map = score tiles
    sta/stb (2 banks each, per-head, double-buffered against their exp) +
    av accumulators (2 banks) + a 2-deep quantum ring (2 banks) = 8.
  - per key-tile-pair window: S_a(n+1) is emitted before AV_a(n)/AV_b(n)
    so exp_a(n+1) can start the moment exp_b(n) retires.
  - qkv/proj GEMM quanta (contract-1024, full-array) are drip-fed into the
    PE slack of each window through the psq ring at ~3 matmuls/window.
Softmax denominator rides the AV matmul via a ones-column widening of V
(AV row 64 = sum_k P); normalize broadcasts the denominator to 64
partitions with a contract-1 ones matmul FIRST, then takes the reciprocal
on all 64 partitions (the old 1-partition reciprocal was 6.5us on DVE).

Engine discipline: PE/ACT/DVE + nc.sync/nc.scalar DMAs only.
"""

import sys

for _p in ("/opt/trn_rl_repo",):
    if _p not in sys.path:
        sys.path.insert(0, _p)

import numpy as np
import ml_dtypes

import bass_rust
import concourse.bass as bass
import concourse.mybir as mybir
import concourse.tile as tile
from concourse.bass_utils import run_bass_kernel_spmd
from concourse.vector_clock import ScopedClock


# --- tail-drain wait splitting -------------------------------------------
# Walrus codegen (CoreV3GenImpl setupSyncWait) rejects CTRL-class
# instructions carrying more than a few sync waits; Tile's kernel-tail drain
# waits on every engine/DMA-queue proc used, which fails codegen.
# Split the waits across the drain plus follow-up sync-engine NOPs emitted
# before the end-of-kernel barrier — semantically identical.
_WAIT_CHUNK = 1


def _split_drain_and_barrier(self, tick_clock, wait_clock):
    drain_inst = self.nc.sync.drain()
    wait_clock.add_sem_waits(
        drain_inst.ins, ScopedClock({None: tick_clock.global_clock})
    )
    si = drain_inst.ins.sync_info
    waits = list(si.on_wait) if si is not None and si.on_wait else []
    if len(waits) > _WAIT_CHUNK:
        si.on_wait = waits[:_WAIT_CHUNK]
        rest = waits[_WAIT_CHUNK:]
        while rest:
            take, rest = rest[:_WAIT_CHUNK], rest[_WAIT_CHUNK:]
            nop = self.nc.sync.nop(nofuse=True, hint="drain_split")
            nop.ins.sync_info = bass_rust.SyncInfo(on_wait=take, on_update=[])
    self.nc.all_engine_barrier()
    popped = self.nc._tile_sem_poison_stack.pop()
    assert popped is self._sem_poison
    self.nc.clear_and_free_semaphores(list(self.sems.allocated().values()))
    self.nc.all_engine_barrier()


tile.TileContext._drain_and_barrier = _split_drain_and_barrier


def _split_multi_waits(nc, limit=1):
    """Hoist all but `limit` sync waits of every instruction onto preceding
    same-engine NOPs (this walrus rejects >1 wait on any instruction)."""
    n = 0
    for f in nc.m.functions:
        for bb in f.blocks:
            new_insts = []
            for ins in bb.instructions:
                si = ins.sync_info
                waits = list(si.on_wait) if si is not None and si.on_wait else []
                if len(waits) > limit and ins.engine not in (
                    None,
                    mybir.EngineType.Unassigned,
                ):
                    for w in waits[:-limit]:
                        nop = mybir.InstNoOp(
                            name=f"{ins.name}.wsplit{n}", ins=[], outs=[]
                        )
                        n += 1
                        nop.engine = ins.engine
                        nop.sync_info = bass_rust.SyncInfo(
                            on_wait=[w], on_update=[]
                        )
                        nc.register_instruction(nop, overwrite=True)
                        new_insts.append(nop)
                    si.on_wait = waits[-limit:]
                new_insts.append(ins)
            bb.instructions = new_insts
    return n


BF16 = mybir.dt.bfloat16
F32 = mybir.dt.float32
NPBF16 = ml_dtypes.bfloat16

B, N, C = 4, 2048, 1024
H, HD = 16, 64
SCALE = HD**-0.5
NQ = N // 2          # local queries per core
P = 128              # partitions
CCH = C // P         # 8 contract chunks
NKT = N // P         # 16 key tiles
HP = H // 2          # 8 head pairs
QG = 512             # matmul free-dim group / query half
KTP = NKT // 2       # 8 key-tile pairs per window loop
VW = HD + 1          # V widened with ones column

_CACHE = {}


def _build():
    nc = bass.Bass()

    xT_d = nc.declare_dram_parameter("xT", [C, N], BF16, isOutput=False)
    wqkT_d = nc.declare_dram_parameter(
        "wqkT", [2 * CCH, P, CCH, P], BF16, isOutput=False
    )  # host pre-packed: [row-group, c-part, c-chunk, row]
    wvT_d = nc.declare_dram_parameter("wvT", [C, C], BF16, isOutput=False)
    wpT_d = nc.declare_dram_parameter("wpT", [C, C], BF16, isOutput=False)
    bias_d = nc.declare_dram_parameter("bias", [1, C], BF16, isOutput=False)
    y_d = nc.declare_dram_parameter("y", [NQ, C], F32, isOutput=True)

    xT_v = xT_d[:].rearrange("(cc p) n -> cc p n", p=P)        # [8,128,2048]
    wvT_v = wvT_d[:].rearrange("(cc p) r -> cc p r", p=P)      # [8,128,1024]
    wpT_v = wpT_d[:].rearrange("(cc p) r -> cc p r", p=P)      # [8,128,1024]

    with tile.TileContext(nc) as tc:
        with (
            tc.tile_pool(name="big", bufs=1) as big,
            tc.tile_pool(name="consts", bufs=1) as consts,
            tc.tile_pool(name="wstream", bufs=3) as wstream,
            tc.tile_pool(name="ptpool", bufs=3) as ptpool,
            tc.tile_pool(name="stashp", bufs=4) as stashp,
            tc.tile_pool(name="recpool", bufs=2) as recpool,
            tc.tile_pool(name="ypool", bufs=2) as ypool,
            tc.tile_pool(name="ps", bufs=1, space="PSUM") as ps,
        ):
            # ---- resident SBUF tensors ----
            xT_sb = big.tile([P, CCH, N], BF16, tag="xT")
            qT_sb = big.tile([P, CCH, NQ], BF16, tag="qT")
            kT_sb = big.tile([P, CCH, N], BF16, tag="kT")
            v_sb = big.tile([P, NKT, H * VW], BF16, tag="v")
            oT_sb = big.tile([P, CCH, NQ], BF16, tag="oT")
            wpT_sb = big.tile([P, CCH, C], BF16, tag="wpT")
            wv_sb = big.tile([P, CCH, C], BF16, tag="wv")
            bias_sb = consts.tile([1, C], BF16, tag="bias")
            ones_sb = consts.tile([P, P], BF16, tag="ones")

            nc.vector.memset(ones_sb[:], 1.0)
            v_ones = v_sb[:].rearrange("p t (h e) -> p t h e", e=VW)[
                :, :, :, HD : HD + 1
            ]
            nc.vector.memset(v_ones, 1.0)

            # ---- startup DMAs, gating-order aware ----
            nc.sync.dma_start(bias_sb[:], bias_d[:])
            # query-half of x^T first (gates Q production), over sync queues
            for cc in range(CCH):
                nc.sync.dma_start(xT_sb[:, cc, 0:NQ], xT_v[cc][:, 0:NQ])
            # key-half + wv over the ACT-frontend queues in parallel
            for cc in range(CCH):
                nc.scalar.dma_start(xT_sb[:, cc, NQ:N], xT_v[cc][:, NQ:N])
            for cc in range(CCH):
                nc.scalar.dma_start(wv_sb[:, cc, :], wvT_v[cc])

            # ---- qkv production quanta (step-granular for interleaving) ----
            def load_qk_slab(rg):
                """DMA 128 rows of w_qk^T (columns rg*128..) as [c-part, cc, row]."""
                wslab = wstream.tile([P, CCH, P], BF16, tag="wqk")
                nc.sync.dma_start(wslab[:], wqkT_d[rg])
                return wslab

            def qk_quantum_steps(rg, wslab, tg):
                """Steps (one matmul each + final copy) producing 128 rows x
                512 tokens of Q^T or K^T through the psq ring."""
                state = {}

                def mk(cc):
                    def step():
                        if cc == 0:
                            state["q"] = ps.tile(
                                [P, QG], F32, tag="q", bufs=2, name=f"qps_{rg}_{tg}"
                            )
                        nc.tensor.matmul(
                            state["q"][:],
                            lhsT=wslab[:, cc, :],
                            rhs=xT_sb[:, cc, tg * QG : (tg + 1) * QG],
                            start=(cc == 0),
                            stop=(cc == CCH - 1),
                        )
                    return step

                def copy_step():
                    dst = qT_sb if rg < CCH else kT_sb
                    ch = rg % CCH
                    nc.vector.tensor_copy(
                        dst[:, ch, tg * QG : (tg + 1) * QG], state["q"][:]
                    )

                return [mk(cc) for cc in range(CCH)] + [copy_step]

            def v_quantum_steps(tc_i, vg):
                """Steps producing V rows for token tile tc_i, head group vg."""
                state = {}

                def mk(cc):
                    def step():
                        if cc == 0:
                            state["q"] = ps.tile(
                                [P, QG], F32, tag="q", bufs=2, name=f"vps_{tc_i}_{vg}"
                            )
                        nc.tensor.matmul(
                            state["q"][:],
                            lhsT=xT_sb[:, cc, tc_i * P : (tc_i + 1) * P],
                            rhs=wv_sb[:, cc, vg * QG : (vg + 1) * QG],
                            start=(cc == 0),
                            stop=(cc == CCH - 1),
                        )
                    return step

                def copy_step():
                    dst = v_sb[
                        :, tc_i, vg * 8 * VW : (vg + 1) * 8 * VW
                    ].rearrange("p (h e) -> p h e", e=VW)[:, :, 0:HD]
                    nc.vector.tensor_copy(
                        dst, state["q"][:].rearrange("p (h e) -> p h e", e=HD)
                    )

                return [mk(cc) for cc in range(CCH)] + [copy_step]

            def run_all(steps):
                for s in steps:
                    s()

            # ---- prologue: pair 0's Q/K rows + first V tiles ----
            slab_q0 = load_qk_slab(0)
            slab_k0 = load_qk_slab(CCH)
            for tg in range(NQ // QG):
                run_all(qk_quantum_steps(0, slab_q0, tg))
            for tg in range(N // QG):
                run_all(qk_quantum_steps(CCH, slab_k0, tg))
            run_all(v_quantum_steps(0, 0))
            run_all(v_quantum_steps(1, 0))

            # ---- normalize helper (deferred off the critical path) ----
            def normalize(h, qh, stash):
                """oT rows of head h, query half qh = stash[0:64] / stash[64].

                Broadcast the denominator row to 64 partitions via a
                contract-1 ones matmul, reciprocal on all 64 partitions,
                then multiply."""
                tagname = f"nrm_{h}_{qh}"
                bc = ps.tile([HD, QG], F32, tag="q", bufs=2, name=f"bc_{tagname}")
                nc.tensor.matmul(
                    bc[:],
                    lhsT=ones_sb[HD : HD + 1, 0:HD],
                    rhs=stash[HD : HD + 1, :],
                    start=True,
                    stop=True,
                )
                recip = recpool.tile([HD, QG], BF16, tag="recip", name=f"rc_{tagname}")
                with nc.allow_low_precision(reason="softmax denom recip"):
                    nc.vector.reciprocal(recip[:], bc[:])
                base = (h % 2) * HD
                nc.vector.tensor_mul(
                    oT_sb[base : base + HD, h // 2, qh * QG : (qh + 1) * QG],
                    stash[0:HD, :],
                    recip[:],
                )

            # vg1 V-tile schedule: first consumed at pair 4; spread over 1-3
            vg1_sched = {1: range(0, 6), 2: range(6, 12), 3: range(12, 16)}

            deferred = []   # pending (h, qh, stash) normalizes
            for hp in range(HP):
                ha, hb = 2 * hp, 2 * hp + 1

                pending = []
                if hp + 1 < HP:
                    nslab_q = load_qk_slab(hp + 1)
                    nslab_k = load_qk_slab(CCH + hp + 1)
                    for tg in range(NQ // QG):
                        pending += qk_quantum_steps(hp + 1, nslab_q, tg)
                    for tg in range(N // QG):
                        pending += qk_quantum_steps(CCH + hp + 1, nslab_k, tg)
                if hp in vg1_sched:
                    for tc_i in vg1_sched[hp]:
                        pending += v_quantum_steps(tc_i, 1)
                if hp == 1:
                    for cc in range(CCH):
                        nc.scalar.dma_start(wpT_sb[:, cc, :], wpT_v[cc])

                # per-pair interleave budget: spread pending evenly over the
                # 16 windows of this pair (qh x ktp), ~ceil division
                n_windows = 2 * KTP
                per_win = (len(pending) + n_windows - 1) // max(1, n_windows)

                def pump(k):
                    for _ in range(min(k, len(pending))):
                        pending.pop(0)()

                for qh in range(2):
                    av = {
                        ha: ps.tile([VW, QG], F32, tag="ava", name=f"av_{ha}_{qh}"),
                        hb: ps.tile([VW, QG], F32, tag="avb", name=f"av_{hb}_{qh}"),
                    }

                    def scores(h, ktp, name):
                        base = (h % 2) * HD
                        st = ps.tile(
                            [P, 2, QG], F32,
                            tag="sta" if h % 2 == 0 else "stb",
                            name=f"st_{name}",
                        )
                        for i in range(2):
                            kt = 2 * ktp + i
                            nc.tensor.matmul(
                                st[:, i, :],
                                lhsT=kT_sb[
                                    base : base + HD, h // 2, kt * P : (kt + 1) * P
                                ],
                                rhs=qT_sb[
                                    base : base + HD, h // 2, qh * QG : (qh + 1) * QG
                                ],
                                start=True,
                                stop=True,
                            )
                        pt = ptpool.tile([P, 2, QG], BF16, tag="pt", name=f"pt_{name}")
                        nc.scalar.activation(
                            pt[:],
                            st[:],
                            mybir.ActivationFunctionType.Exp,
                            scale=float(SCALE),
                        )
                        return pt

                    def av_mms(h, ktp, pt):
                        for i in range(2):
                            kt = 2 * ktp + i
                            nc.tensor.matmul(
                                av[h][:],
                                lhsT=v_sb[:, kt, h * VW : (h + 1) * VW],
                                rhs=pt[:, i, :],
                                start=(ktp == 0 and i == 0),
                                stop=(ktp == KTP - 1 and i == 1),
                            )

                    # window 0: fill the pipe
                    pt_a = scores(ha, 0, f"{ha}_{qh}_0")
                    pt_b = scores(hb, 0, f"{hb}_{qh}_0")

                    for ktp in range(KTP):
                        # pair-0 JIT production of vg0 tiles, one ktp ahead
                        if hp == 0 and qh == 0 and ktp < KTP - 1:
                            run_all(v_quantum_steps(2 * ktp + 2, 0))
                            run_all(v_quantum_steps(2 * ktp + 3, 0))
                        if ktp < KTP - 1:
                            npt_a = scores(ha, ktp + 1, f"{ha}_{qh}_{ktp+1}")
                        av_mms(ha, ktp, pt_a)
                        av_mms(hb, ktp, pt_b)
                        if ktp < KTP - 1:
                            npt_b = scores(hb, ktp + 1, f"{hb}_{qh}_{ktp+1}")
                            pt_a, pt_b = npt_a, npt_b
                        # one deferred normalize piece mid-loop
                        if deferred and ktp in (3, 4):
                            normalize(*deferred.pop(0))
                        pump(per_win)

                    # pair/half boundary: stash unnormalized AV (+ denom row)
                    # so the av PSUM banks recycle for the next half
                    last = hp == HP - 1 and qh == 1
                    for h in (ha, hb):
                        stash = stashp.tile(
                            [VW, QG], BF16, tag="stash", name=f"stash_{h}_{qh}"
                        )
                        nc.vector.tensor_copy(stash[:], av[h][:])
                        if last:
                            normalize(h, qh, stash)
                        else:
                            deferred.append((h, qh, stash))

                pump(len(pending))

            while deferred:
                normalize(*deferred.pop(0))

            # ---- output projection ----
            for tc_i in range(NQ // P):
                for og in range(C // QG):
                    pj = ps.tile(
                        [P, QG], F32, tag="q", bufs=2, name=f"pj_{tc_i}_{og}"
                    )
                    for cc in range(CCH):
                        nc.tensor.matmul(
                            pj[:],
                            lhsT=oT_sb[:, cc, tc_i * P : (tc_i + 1) * P],
                            rhs=wpT_sb[:, cc, og * QG : (og + 1) * QG],
                            start=(cc == 0),
                            stop=False,
                        )
                    nc.tensor.matmul(
                        pj[:],
                        lhsT=ones_sb[0:1, 0:P],
                        rhs=bias_sb[0:1, og * QG : (og + 1) * QG],
                        start=False,
                        stop=True,
                    )
                    y_sb = ypool.tile([P, QG], F32, tag="ysb")
                    nc.vector.tensor_copy(y_sb[:], pj[:])
                    eng = nc.sync if (tc_i + og) % 2 == 0 else nc.scalar
                    eng.dma_start(
                        y_d[tc_i * P : (tc_i + 1) * P, og * QG : (og + 1) * QG],
                        y_sb[:],
                    )
    _split_multi_waits(nc)
    return nc


def get_nc():
    if "nc" not in _CACHE:
        _CACHE["nc"] = _build()
    return _CACHE["nc"]


def make_in_maps(x, w_qkv, w_proj, b_proj):
    x = np.asarray(x, np.float32)
    w_qkv = np.asarray(w_qkv, np.float32)
    w_proj = np.asarray(w_proj, np.float32)
    b_proj = np.asarray(b_proj, np.float32)
    # pre-pack w_qk^T as [row-group, c-part, c-chunk, row] so slab DMAs are
    # fully contiguous per partition
    wqkT = np.ascontiguousarray(
        w_qkv[: 2 * C]
        .T.reshape(CCH, P, 2 * CCH, P)
        .transpose(2, 1, 0, 3)
    ).astype(NPBF16)
    wvT = np.ascontiguousarray(w_qkv[2 * C :].T).astype(NPBF16)
    wpT = np.ascontiguousarray(w_proj.T).astype(NPBF16)
    bias = b_proj.reshape(1, C).astype(NPBF16)
    in_maps = []
    for c in range(8):
        b, s = divmod(c, 2)
        xb = x[b].astype(NPBF16)  # [N, C]
        rolled = np.concatenate(
            [xb[s * NQ : (s + 1) * NQ], xb[(1 - s) * NQ : (2 - s) * NQ]], 0
        )
        xT = np.ascontiguousarray(rolled.T)  # [C, N], local queries first
        in_maps.append({"xT": xT, "wqkT": wqkT, "wvT": wvT, "wpT": wpT, "bias": bias})
    return in_maps


def kernel(x, w_qkv, w_proj, b_proj, _res_out=None):
    nc = get_nc()
    in_maps = make_in_maps(x, w_qkv, w_proj, b_proj)
    res = run_bass_kernel_spmd(nc, in_maps, core_ids=list(range(8)))
    if _res_out is not None:
        _res_out.append(res)

    y = np.empty((B, N, C), np.float32)
    for c in range(8):
        b, s = divmod(c, 2)
        y[b, s * NQ : (s + 1) * NQ] = res.results[c]["y"]
    return y


if __name__ == "__main__":
    rng = np.random.default_rng(0)
    inp = {
        "x": rng.standard_normal((B, N, C), dtype=np.float32),
        "w_qkv": rng.standard_normal((3 * C, C), dtype=np.float32) * C**-0.5,
        "w_proj": rng.standard_normal((C, C), dtype=np.float32) * C**-0.5,
        "b_proj": rng.standard_normal(C, dtype=np.float32) * 0.01,
    }
    y = kernel(**inp)
    print("ran", y.shape, y.dtype)
